# revision 14
# baseline (speedup 1.0000x reference)
"""Trainium2 Bass kernel for nn_Decoder_70781061038951.

Math: for each row b of (B, 65) complex coefficients x = x_real + i*x_imag
(highest degree first), evaluate p(z) at 128 fixed points (64 roots-of-unity
angles on circle radius 1/r and 64 on radius r, r = sqrt(1+sin(pi/64))),
then output m[b, j] = (r^64 * |p(z0_j)| >= |p(z1_j)|) as float32 (B, 64).

Reformulation: polynomial evaluation at fixed points == complex Vandermonde
matmul res = X @ V, decomposed into two real PE matmuls per batch tile:
    psum[b, c] = sum_m XrT[m, b] * W_R[m, c] + XiT[m, b] * W_I[m, c]
with psum column layout [re0(64) | im0(64) | re1(64) | im1(64)] per tile.
Circle-0 columns of V are pre-scaled by s = r^64 and circle-1 by 1/s, so the
compare is  t0 >= s^2 * t1  on squared magnitudes (no sqrt).

Inputs are transposed on the host to [65, B] so the contraction dim lands on
SBUF partitions without any on-device transposes; xr/xi chunks are interleaved
into one array so each chunk is ONE contiguous DMA (the PE Matmult instruction
supports a single sync-wait condition, so every matmul may depend on at most
one new semaphore; tiny warm-up/dummy matmuls pre-absorb the other waits).
Batch is sharded across the 8 NeuronCores (pure data parallel).
"""

import numpy as np

B = 262144
K = 64
KP1 = 65
NCORES = 8
BC = B // NCORES          # 32768 rows per core
P = 128                   # batch tile (psum partitions)
NPTS = 256                # psum cols per tile: re0|im0|re1|im1
TILES = BC // P           # 256
CHUNK = 16                # batch tiles per input DMA chunk
GROUP = 8                 # batch tiles per psum group (4 banks)

_CACHE = {}


def _weights():
    """Host-side Vandermonde blocks W_R, W_I (65, 256) float32 and s^2."""
    r = float(np.sqrt(1.0 + np.sin(np.pi / K)))
    s = r ** K
    ang = 2.0 * np.pi * np.arange(K) / K
    z0 = (1.0 / r) * np.exp(1j * ang)
    z1 = r * np.exp(1j * ang)
    z = np.concatenate([z0, z1])                     # (128,) complex128
    expo = K - np.arange(KP1)                        # (65,) degree of coeff m
    V = z[None, :] ** expo[:, None]                  # (65, 128)
    col_scale = np.where(np.arange(2 * K) < K, s, 1.0 / s)
    Vs = V * col_scale[None, :]
    re, im = Vs.real, Vs.imag
    W_R = np.concatenate(
        [re[:, :K], im[:, :K], re[:, K:], im[:, K:]], axis=1
    ).astype(np.float32)                             # (65, 256)
    W_I = np.concatenate(
        [-im[:, :K], re[:, :K], -im[:, K:], re[:, K:]], axis=1
    ).astype(np.float32)
    return W_R, W_I, float(s * s)


def build_nc(bc=BC, chunk=CHUNK, group=GROUP):
    """Build the single-core Bass program (same program for all 8 cores)."""
    import concourse.bass as bass
    import concourse.tile as tile
    from concourse import mybir

    f32 = mybir.dt.float32
    _, _, s2 = _weights()

    tiles = bc // P
    assert tiles % chunk == 0 and chunk % group == 0
    ccols = chunk * P

    nc = bass.Bass()
    # xb: per-chunk interleave [xr chunk | xi chunk], each chunk one DMA
    xb = nc.declare_dram_parameter("xb", [KP1, 2 * bc], f32, isOutput=False)
    # wb: [W_R | W_I] (65, 512)
    wb = nc.declare_dram_parameter("wb", [KP1, 2 * NPTS], f32, isOutput=False)
    out = nc.declare_dram_parameter("out", [bc, K], f32, isOutput=True)

    with tile.TileContext(nc) as tc:
        with (
            tc.tile_pool(name="consts", bufs=1) as cpool,
            tc.tile_pool(name="xin", bufs=3) as xpool,
            tc.tile_pool(name="ps", bufs=2, space="PSUM") as ppool,
            tc.tile_pool(name="q", bufs=2) as qpool,
            tc.tile_pool(name="m", bufs=2) as mpool,
        ):
            w_sb = cpool.tile([KP1, 2 * NPTS], f32, tag="w")
            nc.sync.dma_start(w_sb[:], wb[:, :])
            wr_sb = w_sb[:, 0:NPTS]
            wi_sb = w_sb[:, NPTS:2 * NPTS]

            # warm-up matmul: absorbs the w-DMA wait so real matmuls never
            # wait on more than one semaphore (PE HW limit: 1 sync wait)
            wps = ppool.tile([1, 2], f32, tag="ps")
            nc.tensor.matmul(
                wps[:], w_sb[:, 0:1], w_sb[:, 0:2], start=True, stop=True
            )

            for c in range(tiles // chunk):
                x_sb = xpool.tile([KP1, 2 * ccols], f32, tag="x")
                nc.sync.dma_start(
                    x_sb[:], xb[:, c * 2 * ccols:(c + 1) * 2 * ccols]
                )

                m_sb = mpool.tile([P, chunk * K], f32, tag="m")

                for g in range(chunk // group):
                    ps = ppool.tile([P, group * NPTS], f32, tag="ps")
                    # dummy matmul: absorbs the psum-reuse (WAR) wait
                    nc.tensor.matmul(
                        ps[0:1, 0:2], w_sb[:, 0:1], w_sb[:, 0:2],
                        start=True, stop=True,
                    )
                    for t in range(group):
                        col = (g * group + t) * P
                        pslice = ps[:, t * NPTS:(t + 1) * NPTS]
                        nc.tensor.matmul(
                            pslice, x_sb[:, col:col + P], wr_sb,
                            start=True, stop=False,
                        )
                        nc.tensor.matmul(
                            pslice, x_sb[:, ccols + col:ccols + col + P], wi_sb,
                            start=False, stop=True,
                        )

                    # q = psum^2 (one big ACT op over the whole group).
                    # dummy copy into a scratch column first: absorbs the
                    # q-slot WAR (DVE) wait so the Square itself carries only
                    # the PE data-ready wait (one sync-wait slot per instr).
                    q_sb = qpool.tile([P, group * NPTS + 1], f32, tag="q")
                    nc.scalar.copy(q_sb[0:1, group * NPTS:], w_sb[0:1, 0:1])
                    nc.scalar.activation(
                        q_sb[:, 0:group * NPTS], ps[:],
                        mybir.ActivationFunctionType.Square,
                    )
                    # tq = re^2 + im^2 ; layout per tile [t0(64) | t1(64)]
                    q4 = q_sb[:, 0:group * NPTS].rearrange(
                        "p (t c i) -> p t c i", t=group, c=2
                    )
                    tq = qpool.tile([P, group * P], f32, tag="tq")
                    nc.vector.tensor_tensor(
                        tq[:], q4[:, :, :, 0:K], q4[:, :, :, K:2 * K],
                        mybir.AluOpType.add,
                    )
                    # m = (s^2 * t1 <= t0)
                    tq3 = tq[:].rearrange("p (t c j) -> p t c j", t=group, c=2)
                    msl = m_sb[:, g * group * K:(g + 1) * group * K]
                    nc.vector.scalar_tensor_tensor(
                        msl.rearrange("p (t j) -> p t j", j=K),
                        tq3[:, :, 1, :], s2, tq3[:, :, 0, :],
                        mybir.AluOpType.mult, mybir.AluOpType.is_le,
                    )

                oview = out[c * chunk * P:(c + 1) * chunk * P, :].rearrange(
                    "(t p) j -> p t j", p=P
                )
                nc.sync.dma_start(oview, m_sb[:].rearrange("p (t j) -> p t j", j=K))

    _reduce_waits(nc)
    _legalize_waits(nc)
    return nc


def _legalize_waits(nc):
    """The TPB instruction encodings carry at most ONE sync-wait. After
    transitive reduction the only residual multi-wait instruction is the
    kernel-tail drain (waits on every DMA lane + engine sem). Split its
    waits onto injected same-engine NoOps placed immediately before it."""
    from concourse import mybir

    for blk in nc.m.functions[0].blocks:
        while True:
            target = None
            for idx, inst in enumerate(blk.instructions):
                si = getattr(inst, "sync_info", None)
                if si is not None and len(si.on_wait) > 1:
                    target = (idx, inst, si)
                    break
            if target is None:
                break
            idx, inst, si = target
            waits = list(si.on_wait)
            for w in waits[:-1]:
                nop = mybir.InstNoOp(
                    name=nc.get_next_instruction_name(), ins=[], outs=[]
                )
                nop.engine = inst.engine
                nop.sync_info = mybir.SyncInfo(on_wait=[w], on_update=[])
                nc.register_instruction(nop)
                blk.instructions.insert(idx, nop)
                idx += 1
            si.on_wait = [waits[-1]]
            inst.sync_info = si


def _reduce_waits(nc, verbose=False):
    """Transitive sync-wait reduction. Tile's add_semaphores pass is minimal
    per-instruction but NOT transitively minimal across engines (documented),
    while the HW instruction encodings have very few wait slots (Matmult: 1).
    Vector-clock pass: a wait (S >= v) is dropped when the knowledge chain of
    the instruction's engine already implies it — via program order on the
    engine's own FIFO queue or via the dispatch-time knowledge of whichever
    instruction posted tick v (completion publishes its knowledge).
    For DMA-lane sems (possible out-of-order completion across rings), tick v
    is conservatively credited with only the knowledge of the k-th posting in
    dispatch order (k = postings needed to reach v)."""
    # engines whose instructions retire in order (self-waits implied)
    fifo_self = {"PE", "Activation", "DVE", "Pool", "SP"}
    # sems that are ever updated by anything other than a plain increment
    # (barrier sems use sem-sub) are non-monotonic: leave them untouched
    nonmono = set()
    for blk in nc.m.functions[0].blocks:
        for inst in blk.instructions:
            si = getattr(inst, "sync_info", None)
            if si is None:
                continue
            for u in si.on_update:
                if u.update_mode not in ("sem-inc", "sem-add-imm") \
                        or u.update_reg is not None:
                    nonmono.add(u.ant_name)
    chain_known = {}   # engine -> {sem_name: guaranteed_value}
    postings = {}      # sem_name -> list of (abs_value, vc_dict)
    cum = {}           # sem_name -> cumulative value
    n_drop = 0
    for blk in nc.m.functions[0].blocks:
        for inst in blk.instructions:
            eng = str(getattr(inst, "engine", "?")).split(".")[-1]
            si = getattr(inst, "sync_info", None)
            if si is None:
                continue
            known = chain_known.setdefault(eng, {})
            # resolve every wait to (sem, value, poster-VC); None = untouchable
            resolved = []
            for w in si.on_wait:
                if (
                    w.wait_mode != "sem-ge-imm"
                    or w.wait_reg is not None
                    or w.ant_name in nonmono
                ):
                    resolved.append((w, None, None))
                    continue
                s, v = w.ant_name, w.wait_value
                vc = None
                for val, pvc in postings.get(s, ()):
                    if val >= v:
                        vc = pvc
                        break
                resolved.append((w, (s, v), vc))
            # drop waits implied by chain knowledge + the other kept waits
            kept = list(range(len(resolved)))
            changed = True
            while changed:
                changed = False
                for i in list(kept):
                    _, sv, _ = resolved[i]
                    if sv is None:
                        continue
                    k = dict(known)
                    for j in kept:
                        if j == i:
                            continue
                        _, svj, vcj = resolved[j]
                        if svj is not None:
                            sj, vj = svj
                            if k.get(sj, -1) < vj:
                                k[sj] = vj
                        if vcj:
                            for ks, kv in vcj.items():
                                if k.get(ks, -1) < kv:
                                    k[ks] = kv
                    if k.get(sv[0], -1) >= sv[1]:
                        kept.remove(i)
                        n_drop += 1
                        changed = True
            # merge kept waits into chain knowledge
            for i in kept:
                _, sv, vc = resolved[i]
                if sv is None:
                    continue
                if vc:
                    for ks, kv in vc.items():
                        if known.get(ks, -1) < kv:
                            known[ks] = kv
                if known.get(sv[0], -1) < sv[1]:
                    known[sv[0]] = sv[1]
            if len(kept) != len(resolved) and \
                    type(inst).__name__ != "InstDrain":
                si.on_wait = [resolved[i][0] for i in kept]
                inst.sync_info = si
            for u in si.on_update:
                if (
                    u.update_mode not in ("sem-inc", "sem-add-imm")
                    or u.update_reg is not None
                    or u.ant_name in nonmono
                ):
                    continue
                s = u.ant_name
                cum[s] = cum.get(s, 0) + u.update_value
                vc_post = dict(known)
                vc_post[s] = max(vc_post.get(s, 0), cum[s])
                postings.setdefault(s, []).append((cum[s], vc_post))
                # NOTE: do NOT credit own ticks to the chain via program
                # order — engines are pipelined, so same-engine RAW hazards
                # legitimately need their self-waits (deep-pipeline rule).
    if verbose:
        print(f"_reduce_waits: dropped {n_drop} waits")
    return n_drop


def _get_nc():
    if "nc" not in _CACHE:
        _CACHE["nc"] = build_nc()
    return _CACHE["nc"]


def _pack_x(xrt_c, xit_c, chunk=CHUNK):
    """Interleave xr/xi chunks: [65, BC]x2 -> [65, 2*BC] per-chunk blocks."""
    kp1, bcc = xrt_c.shape
    ccols = chunk * P
    nchunks = bcc // ccols
    xr3 = xrt_c.reshape(kp1, nchunks, ccols)
    xi3 = xit_c.reshape(kp1, nchunks, ccols)
    return np.stack([xr3, xi3], axis=2).reshape(kp1, 2 * bcc)


def kernel(x_real, x_imag):
    from concourse.bass_utils import run_bass_kernel_spmd

    W_R, W_I, _ = _weights()
    wb = np.concatenate([W_R, W_I], axis=1)          # (65, 512)
    xr = np.asarray(x_real, dtype=np.float32)
    xi = np.asarray(x_imag, dtype=np.float32)
    xrt = np.ascontiguousarray(xr.T)                 # (65, B)
    xit = np.ascontiguousarray(xi.T)

    in_maps = [
        {
            "xb": _pack_x(xrt[:, i * BC:(i + 1) * BC], xit[:, i * BC:(i + 1) * BC]),
            "wb": wb,
        }
        for i in range(NCORES)
    ]
    res = run_bass_kernel_spmd(_get_nc(), in_maps, list(range(NCORES)))
    return np.concatenate(
        [res.results[i]["out"] for i in range(NCORES)], axis=0
    ).astype(np.float32)


# revision 26
# speedup vs baseline: 364.1009x; 364.1009x over previous
"""Trainium2 Bass kernel for nn_Decoder_70781061038951.

Math: for each row b of (B, 65) complex coefficients x = x_real + i*x_imag
(highest degree first), evaluate p(z) at 128 fixed points (64 roots-of-unity
angles on circle radius 1/r and 64 on radius r, r = sqrt(1+sin(pi/64))),
then output m[b, j] = (r^64 * |p(z0_j)| >= |p(z1_j)|) as float32 (B, 64).

Reformulation: polynomial evaluation at fixed points == complex Vandermonde
matmul res = X @ V, decomposed into two real PE matmuls per batch tile:
    psum[b, c] = sum_m XrT[m, b] * W_R[m, c] + XiT[m, b] * W_I[m, c]
with psum column layout [re0(64) | im0(64) | re1(64) | im1(64)] per tile.
Circle-0 columns of V are pre-scaled by s = r^64 and circle-1 by 1/s, so the
compare is  t0 >= s^2 * t1  on squared magnitudes (no sqrt).

Inputs are transposed on the host to [65, B] so the contraction dim lands on
SBUF partitions without any on-device transposes; xr/xi chunks are interleaved
into one array so each chunk is ONE contiguous DMA (the PE Matmult instruction
supports a single sync-wait condition, so every matmul may depend on at most
one new semaphore; tiny warm-up/dummy matmuls pre-absorb the other waits).
Batch is sharded across the 8 NeuronCores (pure data parallel).
"""

import numpy as np

B = 262144
K = 64
KP1 = 65
NCORES = 8
BC = B // NCORES          # 32768 rows per core
P = 128                   # batch tile (psum partitions)
NPTS = 256                # psum cols per tile: re0|im0|re1|im1
TILES = BC // P           # 256
CHUNK = 16                # batch tiles per input DMA chunk
GROUP = 8                 # batch tiles per psum group (4 banks)

_CACHE = {}


def _weights():
    """Host-side Vandermonde blocks W_R, W_I (65, 256) float32 and s^2."""
    r = float(np.sqrt(1.0 + np.sin(np.pi / K)))
    s = r ** K
    ang = 2.0 * np.pi * np.arange(K) / K
    z0 = (1.0 / r) * np.exp(1j * ang)
    z1 = r * np.exp(1j * ang)
    z = np.concatenate([z0, z1])                     # (128,) complex128
    expo = K - np.arange(KP1)                        # (65,) degree of coeff m
    V = z[None, :] ** expo[:, None]                  # (65, 128)
    col_scale = np.where(np.arange(2 * K) < K, s, 1.0 / s)
    Vs = V * col_scale[None, :]
    re, im = Vs.real, Vs.imag
    W_R = np.concatenate(
        [re[:, :K], im[:, :K], re[:, K:], im[:, K:]], axis=1
    ).astype(np.float32)                             # (65, 256)
    W_I = np.concatenate(
        [-im[:, :K], re[:, :K], -im[:, K:], re[:, K:]], axis=1
    ).astype(np.float32)
    return W_R, W_I, float(s * s)


def build_nc(bc=BC, chunk=CHUNK, group=GROUP, reps=1,
             do_mm=True, do_epi=True, do_out=True):
    """Build the single-core Bass program (same program for all 8 cores).

    reps > 1 re-runs the whole computation (idempotent) inside one NEFF —
    used to measure device time differentially under the axon dispatch
    overhead: T(R) - T(1) = (R-1) * device_time."""
    import concourse.bass as bass
    import concourse.tile as tile
    from concourse import mybir

    f32 = mybir.dt.float32
    _, _, s2 = _weights()

    tiles = bc // P
    assert tiles % chunk == 0 and chunk % group == 0
    ccols = chunk * P

    nc = bass.Bass()
    # xb: per-chunk interleave [xr chunk | xi chunk], each chunk one DMA
    xb = nc.declare_dram_parameter("xb", [KP1, 2 * bc], f32, isOutput=False)
    # wb: [W_R | W_I] (65, 512)
    wb = nc.declare_dram_parameter("wb", [KP1, 2 * NPTS], f32, isOutput=False)
    out = nc.declare_dram_parameter("out", [bc, K], f32, isOutput=True)

    with tile.TileContext(nc) as tc:
        with (
            tc.tile_pool(name="consts", bufs=1) as cpool,
            tc.tile_pool(name="xin", bufs=3) as xpool,
            tc.tile_pool(name="ps", bufs=2, space="PSUM") as ppool,
            tc.tile_pool(name="q", bufs=2) as qpool,
            tc.tile_pool(name="m", bufs=2) as mpool,
        ):
            w_sb = cpool.tile([KP1, 2 * NPTS], f32, tag="w")
            nc.sync.dma_start(w_sb[:], wb[:, :])
            wr_sb = w_sb[:, 0:NPTS]
            wi_sb = w_sb[:, NPTS:2 * NPTS]

            # warm-up matmul: absorbs the w-DMA wait so real matmuls never
            # wait on more than one semaphore (PE HW limit: 1 sync wait)
            wps = ppool.tile([1, 2], f32, tag="ps")
            nc.tensor.matmul(
                wps[:], w_sb[:, 0:1], w_sb[:, 0:2], start=True, stop=True
            )

            for c in [cc for _ in range(reps) for cc in range(tiles // chunk)]:
                x_sb = xpool.tile([KP1, 2 * ccols], f32, tag="x")
                nc.sync.dma_start(
                    x_sb[:], xb[:, c * 2 * ccols:(c + 1) * 2 * ccols]
                )

                m_sb = mpool.tile([P, chunk * K], f32, tag="m")

                for g in range(chunk // group) if do_mm else []:
                    ps = ppool.tile([P, group * NPTS], f32, tag="ps")
                    # dummy matmul: absorbs the psum-reuse (WAR) wait
                    nc.tensor.matmul(
                        ps[0:1, 0:2], w_sb[:, 0:1], w_sb[:, 0:2],
                        start=True, stop=True,
                    )
                    for t in range(group):
                        col = (g * group + t) * P
                        pslice = ps[:, t * NPTS:(t + 1) * NPTS]
                        nc.tensor.matmul(
                            pslice, x_sb[:, col:col + P], wr_sb,
                            start=True, stop=False,
                        )
                        nc.tensor.matmul(
                            pslice, x_sb[:, ccols + col:ccols + col + P], wi_sb,
                            start=False, stop=True,
                        )

                    if not do_epi:
                        continue
                    # q = psum^2 (one big ACT op over the whole group).
                    # dummy copy into a scratch column first: absorbs the
                    # q-slot WAR (DVE) wait so the Square itself carries only
                    # the PE data-ready wait (one sync-wait slot per instr).
                    q_sb = qpool.tile([P, group * NPTS + 1], f32, tag="q")
                    nc.scalar.copy(q_sb[0:1, group * NPTS:], w_sb[0:1, 0:1])
                    nc.scalar.activation(
                        q_sb[:, 0:group * NPTS], ps[:],
                        mybir.ActivationFunctionType.Square,
                    )
                    # tq = re^2 + im^2 ; layout per tile [t0(64) | t1(64)]
                    q4 = q_sb[:, 0:group * NPTS].rearrange(
                        "p (t c i) -> p t c i", t=group, c=2
                    )
                    tq = qpool.tile([P, group * P], f32, tag="tq")
                    nc.vector.tensor_tensor(
                        tq[:], q4[:, :, :, 0:K], q4[:, :, :, K:2 * K],
                        mybir.AluOpType.add,
                    )
                    # m = (s^2 * t1 <= t0)
                    tq3 = tq[:].rearrange("p (t c j) -> p t c j", t=group, c=2)
                    msl = m_sb[:, g * group * K:(g + 1) * group * K]
                    nc.vector.scalar_tensor_tensor(
                        msl.rearrange("p (t j) -> p t j", j=K),
                        tq3[:, :, 1, :], s2, tq3[:, :, 0, :],
                        mybir.AluOpType.mult, mybir.AluOpType.is_le,
                    )

                if do_out and do_mm and do_epi:
                    oview = out[c * chunk * P:(c + 1) * chunk * P, :].rearrange(
                        "(t p) j -> p t j", p=P
                    )
                    nc.sync.dma_start(
                        oview, m_sb[:].rearrange("p (t j) -> p t j", j=K)
                    )

    _reduce_waits(nc)
    _legalize_waits(nc)
    return nc


def _legalize_waits(nc):
    """The TPB instruction encodings carry at most ONE sync-wait. After
    transitive reduction the only residual multi-wait instruction is the
    kernel-tail drain (waits on every DMA lane + engine sem). Split its
    waits onto injected same-engine NoOps placed immediately before it."""
    from concourse import mybir

    for blk in nc.m.functions[0].blocks:
        while True:
            target = None
            for idx, inst in enumerate(blk.instructions):
                si = getattr(inst, "sync_info", None)
                if si is not None and len(si.on_wait) > 1:
                    target = (idx, inst, si)
                    break
            if target is None:
                break
            idx, inst, si = target
            waits = list(si.on_wait)
            for w in waits[:-1]:
                nop = mybir.InstNoOp(
                    name=nc.get_next_instruction_name(), ins=[], outs=[]
                )
                nop.engine = inst.engine
                nop.sync_info = mybir.SyncInfo(on_wait=[w], on_update=[])
                nc.register_instruction(nop)
                blk.instructions.insert(idx, nop)
                idx += 1
            si.on_wait = [waits[-1]]
            inst.sync_info = si


def _reduce_waits(nc, verbose=False):
    """Transitive sync-wait reduction. Tile's add_semaphores pass is minimal
    per-instruction but NOT transitively minimal across engines (documented),
    while the HW instruction encodings have very few wait slots (Matmult: 1).
    Vector-clock pass: a wait (S >= v) is dropped when the knowledge chain of
    the instruction's engine already implies it — via program order on the
    engine's own FIFO queue or via the dispatch-time knowledge of whichever
    instruction posted tick v (completion publishes its knowledge).
    For DMA-lane sems (possible out-of-order completion across rings), tick v
    is conservatively credited with only the knowledge of the k-th posting in
    dispatch order (k = postings needed to reach v)."""
    # engines whose instructions retire in order (self-waits implied)
    fifo_self = {"PE", "Activation", "DVE", "Pool", "SP"}
    # sems that are ever updated by anything other than a plain increment
    # (barrier sems use sem-sub) are non-monotonic: leave them untouched
    nonmono = set()
    for blk in nc.m.functions[0].blocks:
        for inst in blk.instructions:
            si = getattr(inst, "sync_info", None)
            if si is None:
                continue
            for u in si.on_update:
                if u.update_mode not in ("sem-inc", "sem-add-imm") \
                        or u.update_reg is not None:
                    nonmono.add(u.ant_name)
    chain_known = {}   # engine -> {sem_name: guaranteed_value}
    postings = {}      # sem_name -> list of (abs_value, vc_dict)
    cum = {}           # sem_name -> cumulative value
    n_drop = 0
    for blk in nc.m.functions[0].blocks:
        for inst in blk.instructions:
            eng = str(getattr(inst, "engine", "?")).split(".")[-1]
            si = getattr(inst, "sync_info", None)
            if si is None:
                continue
            known = chain_known.setdefault(eng, {})
            # resolve every wait to (sem, value, poster-VC); None = untouchable
            resolved = []
            for w in si.on_wait:
                if (
                    w.wait_mode != "sem-ge-imm"
                    or w.wait_reg is not None
                    or w.ant_name in nonmono
                ):
                    resolved.append((w, None, None))
                    continue
                s, v = w.ant_name, w.wait_value
                vc = None
                for val, pvc in postings.get(s, ()):
                    if val >= v:
                        vc = pvc
                        break
                resolved.append((w, (s, v), vc))
            # drop waits implied by chain knowledge + the other kept waits
            kept = list(range(len(resolved)))
            changed = True
            while changed:
                changed = False
                for i in list(kept):
                    _, sv, _ = resolved[i]
                    if sv is None:
                        continue
                    k = dict(known)
                    for j in kept:
                        if j == i:
                            continue
                        _, svj, vcj = resolved[j]
                        if svj is not None:
                            sj, vj = svj
                            if k.get(sj, -1) < vj:
                                k[sj] = vj
                        if vcj:
                            for ks, kv in vcj.items():
                                if k.get(ks, -1) < kv:
                                    k[ks] = kv
                    if k.get(sv[0], -1) >= sv[1]:
                        kept.remove(i)
                        n_drop += 1
                        changed = True
            # merge kept waits into chain knowledge
            for i in kept:
                _, sv, vc = resolved[i]
                if sv is None:
                    continue
                if vc:
                    for ks, kv in vc.items():
                        if known.get(ks, -1) < kv:
                            known[ks] = kv
                if known.get(sv[0], -1) < sv[1]:
                    known[sv[0]] = sv[1]
            if len(kept) != len(resolved) and \
                    type(inst).__name__ != "InstDrain":
                si.on_wait = [resolved[i][0] for i in kept]
                inst.sync_info = si
            for u in si.on_update:
                if (
                    u.update_mode not in ("sem-inc", "sem-add-imm")
                    or u.update_reg is not None
                    or u.ant_name in nonmono
                ):
                    continue
                s = u.ant_name
                cum[s] = cum.get(s, 0) + u.update_value
                vc_post = dict(known)
                vc_post[s] = max(vc_post.get(s, 0), cum[s])
                postings.setdefault(s, []).append((cum[s], vc_post))
                # NOTE: do NOT credit own ticks to the chain via program
                # order — engines are pipelined, so same-engine RAW hazards
                # legitimately need their self-waits (deep-pipeline rule).
    if verbose:
        print(f"_reduce_waits: dropped {n_drop} waits")
    return n_drop


def _weights2():
    """bf16 hi/lo split weights for the K=128-packed scheme.

    Coefficient rows m=1..64 go into the matmul (the m=0 row of V is the
    constant r_c^64 per circle — after column scaling its coefficient is
    exactly 1.0 on re-columns for x_real[:,0] and on im-columns for
    x_imag[:,0], handled by a tiny K=4 matmul with an exact 0/1 pattern).
    Returns WHI/WLO (128, 256) bf16-in-f32, E4 (4, 256), s^2."""
    import ml_dtypes

    W_R, W_I, s2 = _weights()
    # rows m=1..64 stacked: [W_R[1:65] ; W_I[1:65]] -> (128, 256)
    w = np.concatenate([W_R[1:], W_I[1:]], axis=0).astype(np.float32)
    whi = w.astype(ml_dtypes.bfloat16)
    wlo = (w - whi.astype(np.float32)).astype(ml_dtypes.bfloat16)
    # x0 pattern: re-columns get xr0 (rows 0,1 = hi,lo), im-columns get xi0
    re_col = ((np.arange(NPTS) % 128) < K).astype(np.float32)
    e4 = np.stack([re_col, re_col, 1.0 - re_col, 1.0 - re_col]).astype(
        ml_dtypes.bfloat16
    )                                                    # (4, 256)
    return whi, wlo, e4, s2


def build_nc2(bc=BC, chunk=CHUNK, group=GROUP, reps=1,
              terms=4, do_epi=True, out_u8=False, psbufs=2, qbufs=2, mbufs=2):
    """bf16 split-precision variant: per tile, 4 accumulating matmuls
    (K=4 x0-term, hi@Whi, hi@Wlo, lo@Whi) at 1 cycle/row instead of fp32's
    4 cycles/row. Inputs are host-split into bf16 hi+lo (same total bytes
    as fp32) and packed [xr(64);xi(64)] so DMAs span all 128 partitions."""
    import concourse.bass as bass
    import concourse.tile as tile
    from concourse import mybir

    f32 = mybir.dt.float32
    bf16 = mybir.dt.bfloat16
    _, _, _, s2 = _weights2()

    tiles = bc // P
    assert tiles % chunk == 0 and chunk % group == 0
    ccols = chunk * P

    nc = bass.Bass()
    # per-chunk interleave [hi-chunk(128 rows) | lo-chunk], bf16
    xs = nc.declare_dram_parameter("xs", [P, 2 * bc], bf16, isOutput=False)
    # x0 splits: rows xr0_hi, xr0_lo, xi0_hi, xi0_lo
    x0 = nc.declare_dram_parameter("x0", [4, bc], bf16, isOutput=False)
    # [WHI | WLO] (128, 512) bf16
    wb = nc.declare_dram_parameter("wb", [P, 2 * NPTS], bf16, isOutput=False)
    e4 = nc.declare_dram_parameter("e4", [4, NPTS], bf16, isOutput=False)
    out_dt = mybir.dt.uint8 if out_u8 else f32
    out = nc.declare_dram_parameter("out", [bc, K], out_dt, isOutput=True)

    with tile.TileContext(nc) as tc:
        with (
            tc.tile_pool(name="consts", bufs=1) as cpool,
            tc.tile_pool(name="xin", bufs=3) as xpool,
            tc.tile_pool(name="x0in", bufs=3) as zpool,
            tc.tile_pool(name="ps", bufs=psbufs, space="PSUM") as ppool,
            tc.tile_pool(name="q", bufs=qbufs) as qpool,
            tc.tile_pool(name="m", bufs=mbufs) as mpool,
        ):
            w_sb = cpool.tile([P, 2 * NPTS], bf16, tag="w")
            nc.sync.dma_start(w_sb[:], wb[:, :])
            e4_sb = cpool.tile([4, NPTS], bf16, tag="e4")
            nc.sync.dma_start(e4_sb[:], e4[:, :])
            whi_sb = w_sb[:, 0:NPTS]
            wlo_sb = w_sb[:, NPTS:2 * NPTS]

            # warm-up matmuls: absorb the const-DMA waits (1-wait HW limit)
            wps = ppool.tile([1, 2], f32, tag="ps")
            nc.tensor.matmul(
                wps[:], w_sb[:, 0:1], w_sb[:, 0:2], start=True, stop=True
            )
            nc.tensor.matmul(
                wps[:], e4_sb[:, 0:1], e4_sb[:, 0:2], start=True, stop=True
            )

            for c in [cc for _ in range(reps) for cc in range(tiles // chunk)]:
                x_sb = xpool.tile([P, 2 * ccols], bf16, tag="x")
                nc.sync.dma_start(
                    x_sb[:], xs[:, c * 2 * ccols:(c + 1) * 2 * ccols]
                )
                z_sb = zpool.tile([4, ccols], bf16, tag="z")
                nc.sync.dma_start(z_sb[:], x0[:, c * ccols:(c + 1) * ccols])

                m_sb = mpool.tile([P, chunk * K], out_dt, tag="m")

                for g in range(chunk // group):
                    ps = ppool.tile([P, group * NPTS], f32, tag="ps")
                    # dummy: absorbs the psum-reuse (WAR) wait
                    nc.tensor.matmul(
                        ps[0:1, 0:2], w_sb[:, 0:1], w_sb[:, 0:2],
                        start=True, stop=True,
                    )
                    for t in range(group):
                        col = (g * group + t) * P
                        pslice = ps[:, t * NPTS:(t + 1) * NPTS]
                        xhi = x_sb[:, col:col + P]
                        xlo = x_sb[:, ccols + col:ccols + col + P]
                        mms = [(z_sb[:, col:col + P], e4_sb[:]),
                               (xhi, whi_sb), (xhi, wlo_sb), (xlo, whi_sb)]
                        mms = mms[4 - terms:]
                        for i, (lhs, rhs) in enumerate(mms):
                            nc.tensor.matmul(
                                pslice, lhs, rhs,
                                start=(i == 0), stop=(i == len(mms) - 1),
                            )

                    # epilogue identical to v1
                    if not do_epi:
                        continue
                    q_sb = qpool.tile([P, group * NPTS + 1], f32, tag="q")
                    nc.scalar.copy(q_sb[0:1, group * NPTS:], w_sb[0:1, 0:1])
                    nc.scalar.activation(
                        q_sb[:, 0:group * NPTS], ps[:],
                        mybir.ActivationFunctionType.Square,
                    )
                    q4 = q_sb[:, 0:group * NPTS].rearrange(
                        "p (t c i) -> p t c i", t=group, c=2
                    )
                    tq = qpool.tile([P, group * P], f32, tag="tq")
                    nc.vector.tensor_tensor(
                        tq[:], q4[:, :, :, 0:K], q4[:, :, :, K:2 * K],
                        mybir.AluOpType.add,
                    )
                    tq3 = tq[:].rearrange("p (t c j) -> p t c j", t=group, c=2)
                    msl = m_sb[:, g * group * K:(g + 1) * group * K]
                    nc.vector.scalar_tensor_tensor(
                        msl.rearrange("p (t j) -> p t j", j=K),
                        tq3[:, :, 1, :], s2, tq3[:, :, 0, :],
                        mybir.AluOpType.mult, mybir.AluOpType.is_le,
                    )

                if do_epi:
                    oview = out[c * chunk * P:(c + 1) * chunk * P, :].rearrange(
                        "(t p) j -> p t j", p=P
                    )
                    nc.sync.dma_start(
                        oview, m_sb[:].rearrange("p (t j) -> p t j", j=K)
                    )

    _reduce_waits(nc)
    _legalize_waits(nc)
    return nc


def _pack_x2(xr, xi, chunk=CHUNK):
    """Host prep for build_nc2 from natural (bc, 65) f32 slices.

    Returns xs (128, 2*bc) bf16 with per-chunk [hi | lo] blocks where
    rows 0:64 = xr coeffs m=1..64 transposed, rows 64:128 = xi coeffs;
    and x0 (4, bc) bf16 rows xr0_hi, xr0_lo, xi0_hi, xi0_lo."""
    import ml_dtypes

    bf = ml_dtypes.bfloat16
    bcc = xr.shape[0]
    coef = np.concatenate([xr[:, 1:].T, xi[:, 1:].T], axis=0)  # (128, bc) f32
    hi = coef.astype(bf)
    lo = (coef - hi.astype(np.float32)).astype(bf)
    ccols = chunk * P
    nch = bcc // ccols
    hi3 = hi.reshape(P, nch, ccols)
    lo3 = lo.reshape(P, nch, ccols)
    xs = np.stack([hi3, lo3], axis=2).reshape(P, 2 * bcc)

    x0c = np.stack([xr[:, 0], xi[:, 0]]).astype(np.float32)    # (2, bc)
    h0 = x0c.astype(bf)
    l0 = (x0c - h0.astype(np.float32)).astype(bf)
    x0 = np.stack([h0[0], l0[0], h0[1], l0[1]])                # (4, bc)
    return np.ascontiguousarray(xs), np.ascontiguousarray(x0)


def _get_nc():
    if "nc" not in _CACHE:
        _CACHE["nc"] = build_nc2(out_u8=True, qbufs=3, mbufs=3)
    return _CACHE["nc"]


def _pack_x(xrt_c, xit_c, chunk=CHUNK):
    """Interleave xr/xi chunks: [65, BC]x2 -> [65, 2*BC] per-chunk blocks."""
    kp1, bcc = xrt_c.shape
    ccols = chunk * P
    nchunks = bcc // ccols
    xr3 = xrt_c.reshape(kp1, nchunks, ccols)
    xi3 = xit_c.reshape(kp1, nchunks, ccols)
    return np.stack([xr3, xi3], axis=2).reshape(kp1, 2 * bcc)


def make_in_maps(x_real, x_imag):
    whi, wlo, e4, _ = _weights2()
    wb = np.ascontiguousarray(
        np.concatenate([whi, wlo], axis=1)
    )                                                # (128, 512) bf16
    xr = np.asarray(x_real, dtype=np.float32)
    xi = np.asarray(x_imag, dtype=np.float32)
    in_maps = []
    for i in range(NCORES):
        xs, x0 = _pack_x2(xr[i * BC:(i + 1) * BC], xi[i * BC:(i + 1) * BC])
        in_maps.append({"xs": xs, "x0": x0, "wb": wb, "e4": e4})
    return in_maps


def kernel(x_real, x_imag):
    from concourse.bass_utils import run_bass_kernel_spmd

    in_maps = make_in_maps(x_real, x_imag)
    res = run_bass_kernel_spmd(_get_nc(), in_maps, list(range(NCORES)))
    # device emits uint8 0/1; the reference dtype is float32
    return np.concatenate(
        [res.results[i]["out"] for i in range(NCORES)], axis=0
    ).astype(np.float32)


# revision 27
# speedup vs baseline: 366.1251x; 1.0056x over previous
"""Trainium2 Bass kernel for nn_Decoder_70781061038951.

Math: for each row b of (B, 65) complex coefficients x = x_real + i*x_imag
(highest degree first), evaluate p(z) at 128 fixed points (64 roots-of-unity
angles on circle radius 1/r and 64 on radius r, r = sqrt(1+sin(pi/64))),
then output m[b, j] = (r^64 * |p(z0_j)| >= |p(z1_j)|) as float32 (B, 64).

Reformulation: polynomial evaluation at fixed points == complex Vandermonde
matmul res = X @ V, decomposed into two real PE matmuls per batch tile:
    psum[b, c] = sum_m XrT[m, b] * W_R[m, c] + XiT[m, b] * W_I[m, c]
with psum column layout [re0(64) | im0(64) | re1(64) | im1(64)] per tile.
Circle-0 columns of V are pre-scaled by s = r^64 and circle-1 by 1/s, so the
compare is  t0 >= s^2 * t1  on squared magnitudes (no sqrt).

Inputs are transposed on the host to [65, B] so the contraction dim lands on
SBUF partitions without any on-device transposes; xr/xi chunks are interleaved
into one array so each chunk is ONE contiguous DMA (the PE Matmult instruction
supports a single sync-wait condition, so every matmul may depend on at most
one new semaphore; tiny warm-up/dummy matmuls pre-absorb the other waits).
Batch is sharded across the 8 NeuronCores (pure data parallel).
"""

import numpy as np

B = 262144
K = 64
KP1 = 65
NCORES = 8
BC = B // NCORES          # 32768 rows per core
P = 128                   # batch tile (psum partitions)
NPTS = 256                # psum cols per tile: re0|im0|re1|im1
TILES = BC // P           # 256
CHUNK = 16                # batch tiles per input DMA chunk
GROUP = 8                 # batch tiles per psum group (4 banks)

_CACHE = {}


def _weights():
    """Host-side Vandermonde blocks W_R, W_I (65, 256) float32 and s^2."""
    r = float(np.sqrt(1.0 + np.sin(np.pi / K)))
    s = r ** K
    ang = 2.0 * np.pi * np.arange(K) / K
    z0 = (1.0 / r) * np.exp(1j * ang)
    z1 = r * np.exp(1j * ang)
    z = np.concatenate([z0, z1])                     # (128,) complex128
    expo = K - np.arange(KP1)                        # (65,) degree of coeff m
    V = z[None, :] ** expo[:, None]                  # (65, 128)
    col_scale = np.where(np.arange(2 * K) < K, s, 1.0 / s)
    Vs = V * col_scale[None, :]
    re, im = Vs.real, Vs.imag
    W_R = np.concatenate(
        [re[:, :K], im[:, :K], re[:, K:], im[:, K:]], axis=1
    ).astype(np.float32)                             # (65, 256)
    W_I = np.concatenate(
        [-im[:, :K], re[:, :K], -im[:, K:], re[:, K:]], axis=1
    ).astype(np.float32)
    return W_R, W_I, float(s * s)


def build_nc(bc=BC, chunk=CHUNK, group=GROUP, reps=1,
             do_mm=True, do_epi=True, do_out=True):
    """Build the single-core Bass program (same program for all 8 cores).

    reps > 1 re-runs the whole computation (idempotent) inside one NEFF —
    used to measure device time differentially under the axon dispatch
    overhead: T(R) - T(1) = (R-1) * device_time."""
    import concourse.bass as bass
    import concourse.tile as tile
    from concourse import mybir

    f32 = mybir.dt.float32
    _, _, s2 = _weights()

    tiles = bc // P
    assert tiles % chunk == 0 and chunk % group == 0
    ccols = chunk * P

    nc = bass.Bass()
    # xb: per-chunk interleave [xr chunk | xi chunk], each chunk one DMA
    xb = nc.declare_dram_parameter("xb", [KP1, 2 * bc], f32, isOutput=False)
    # wb: [W_R | W_I] (65, 512)
    wb = nc.declare_dram_parameter("wb", [KP1, 2 * NPTS], f32, isOutput=False)
    out = nc.declare_dram_parameter("out", [bc, K], f32, isOutput=True)

    with tile.TileContext(nc) as tc:
        with (
            tc.tile_pool(name="consts", bufs=1) as cpool,
            tc.tile_pool(name="xin", bufs=3) as xpool,
            tc.tile_pool(name="ps", bufs=2, space="PSUM") as ppool,
            tc.tile_pool(name="q", bufs=2) as qpool,
            tc.tile_pool(name="m", bufs=2) as mpool,
        ):
            w_sb = cpool.tile([KP1, 2 * NPTS], f32, tag="w")
            nc.sync.dma_start(w_sb[:], wb[:, :])
            wr_sb = w_sb[:, 0:NPTS]
            wi_sb = w_sb[:, NPTS:2 * NPTS]

            # warm-up matmul: absorbs the w-DMA wait so real matmuls never
            # wait on more than one semaphore (PE HW limit: 1 sync wait)
            wps = ppool.tile([1, 2], f32, tag="ps")
            nc.tensor.matmul(
                wps[:], w_sb[:, 0:1], w_sb[:, 0:2], start=True, stop=True
            )

            for c in [cc for _ in range(reps) for cc in range(tiles // chunk)]:
                x_sb = xpool.tile([KP1, 2 * ccols], f32, tag="x")
                nc.sync.dma_start(
                    x_sb[:], xb[:, c * 2 * ccols:(c + 1) * 2 * ccols]
                )

                m_sb = mpool.tile([P, chunk * K], f32, tag="m")

                for g in range(chunk // group) if do_mm else []:
                    ps = ppool.tile([P, group * NPTS], f32, tag="ps")
                    # dummy matmul: absorbs the psum-reuse (WAR) wait
                    nc.tensor.matmul(
                        ps[0:1, 0:2], w_sb[:, 0:1], w_sb[:, 0:2],
                        start=True, stop=True,
                    )
                    for t in range(group):
                        col = (g * group + t) * P
                        pslice = ps[:, t * NPTS:(t + 1) * NPTS]
                        nc.tensor.matmul(
                            pslice, x_sb[:, col:col + P], wr_sb,
                            start=True, stop=False,
                        )
                        nc.tensor.matmul(
                            pslice, x_sb[:, ccols + col:ccols + col + P], wi_sb,
                            start=False, stop=True,
                        )

                    if not do_epi:
                        continue
                    # q = psum^2 (one big ACT op over the whole group).
                    # dummy copy into a scratch column first: absorbs the
                    # q-slot WAR (DVE) wait so the Square itself carries only
                    # the PE data-ready wait (one sync-wait slot per instr).
                    q_sb = qpool.tile([P, group * NPTS + 1], f32, tag="q")
                    nc.scalar.copy(q_sb[0:1, group * NPTS:], w_sb[0:1, 0:1])
                    nc.scalar.activation(
                        q_sb[:, 0:group * NPTS], ps[:],
                        mybir.ActivationFunctionType.Square,
                    )
                    # tq = re^2 + im^2 ; layout per tile [t0(64) | t1(64)]
                    q4 = q_sb[:, 0:group * NPTS].rearrange(
                        "p (t c i) -> p t c i", t=group, c=2
                    )
                    tq = qpool.tile([P, group * P], f32, tag="tq")
                    nc.vector.tensor_tensor(
                        tq[:], q4[:, :, :, 0:K], q4[:, :, :, K:2 * K],
                        mybir.AluOpType.add,
                    )
                    # m = (s^2 * t1 <= t0)
                    tq3 = tq[:].rearrange("p (t c j) -> p t c j", t=group, c=2)
                    msl = m_sb[:, g * group * K:(g + 1) * group * K]
                    nc.vector.scalar_tensor_tensor(
                        msl.rearrange("p (t j) -> p t j", j=K),
                        tq3[:, :, 1, :], s2, tq3[:, :, 0, :],
                        mybir.AluOpType.mult, mybir.AluOpType.is_le,
                    )

                if do_out and do_mm and do_epi:
                    oview = out[c * chunk * P:(c + 1) * chunk * P, :].rearrange(
                        "(t p) j -> p t j", p=P
                    )
                    nc.sync.dma_start(
                        oview, m_sb[:].rearrange("p (t j) -> p t j", j=K)
                    )

    _reduce_waits(nc)
    _legalize_waits(nc)
    return nc


def _legalize_waits(nc):
    """The TPB instruction encodings carry at most ONE sync-wait. After
    transitive reduction the only residual multi-wait instruction is the
    kernel-tail drain (waits on every DMA lane + engine sem). Split its
    waits onto injected same-engine NoOps placed immediately before it."""
    from concourse import mybir

    for blk in nc.m.functions[0].blocks:
        while True:
            target = None
            for idx, inst in enumerate(blk.instructions):
                si = getattr(inst, "sync_info", None)
                if si is not None and len(si.on_wait) > 1:
                    target = (idx, inst, si)
                    break
            if target is None:
                break
            idx, inst, si = target
            waits = list(si.on_wait)
            for w in waits[:-1]:
                nop = mybir.InstNoOp(
                    name=nc.get_next_instruction_name(), ins=[], outs=[]
                )
                nop.engine = inst.engine
                nop.sync_info = mybir.SyncInfo(on_wait=[w], on_update=[])
                nc.register_instruction(nop)
                blk.instructions.insert(idx, nop)
                idx += 1
            si.on_wait = [waits[-1]]
            inst.sync_info = si


def _reduce_waits(nc, verbose=False):
    """Transitive sync-wait reduction. Tile's add_semaphores pass is minimal
    per-instruction but NOT transitively minimal across engines (documented),
    while the HW instruction encodings have very few wait slots (Matmult: 1).
    Vector-clock pass: a wait (S >= v) is dropped when the knowledge chain of
    the instruction's engine already implies it — via program order on the
    engine's own FIFO queue or via the dispatch-time knowledge of whichever
    instruction posted tick v (completion publishes its knowledge).
    For DMA-lane sems (possible out-of-order completion across rings), tick v
    is conservatively credited with only the knowledge of the k-th posting in
    dispatch order (k = postings needed to reach v)."""
    # engines whose instructions retire in order (self-waits implied)
    fifo_self = {"PE", "Activation", "DVE", "Pool", "SP"}
    # sems that are ever updated by anything other than a plain increment
    # (barrier sems use sem-sub) are non-monotonic: leave them untouched
    nonmono = set()
    for blk in nc.m.functions[0].blocks:
        for inst in blk.instructions:
            si = getattr(inst, "sync_info", None)
            if si is None:
                continue
            for u in si.on_update:
                if u.update_mode not in ("sem-inc", "sem-add-imm") \
                        or u.update_reg is not None:
                    nonmono.add(u.ant_name)
    chain_known = {}   # engine -> {sem_name: guaranteed_value}
    postings = {}      # sem_name -> list of (abs_value, vc_dict)
    cum = {}           # sem_name -> cumulative value
    n_drop = 0
    for blk in nc.m.functions[0].blocks:
        for inst in blk.instructions:
            eng = str(getattr(inst, "engine", "?")).split(".")[-1]
            si = getattr(inst, "sync_info", None)
            if si is None:
                continue
            known = chain_known.setdefault(eng, {})
            # resolve every wait to (sem, value, poster-VC); None = untouchable
            resolved = []
            for w in si.on_wait:
                if (
                    w.wait_mode != "sem-ge-imm"
                    or w.wait_reg is not None
                    or w.ant_name in nonmono
                ):
                    resolved.append((w, None, None))
                    continue
                s, v = w.ant_name, w.wait_value
                vc = None
                for val, pvc in postings.get(s, ()):
                    if val >= v:
                        vc = pvc
                        break
                resolved.append((w, (s, v), vc))
            # drop waits implied by chain knowledge + the other kept waits
            kept = list(range(len(resolved)))
            changed = True
            while changed:
                changed = False
                for i in list(kept):
                    _, sv, _ = resolved[i]
                    if sv is None:
                        continue
                    k = dict(known)
                    for j in kept:
                        if j == i:
                            continue
                        _, svj, vcj = resolved[j]
                        if svj is not None:
                            sj, vj = svj
                            if k.get(sj, -1) < vj:
                                k[sj] = vj
                        if vcj:
                            for ks, kv in vcj.items():
                                if k.get(ks, -1) < kv:
                                    k[ks] = kv
                    if k.get(sv[0], -1) >= sv[1]:
                        kept.remove(i)
                        n_drop += 1
                        changed = True
            # merge kept waits into chain knowledge
            for i in kept:
                _, sv, vc = resolved[i]
                if sv is None:
                    continue
                if vc:
                    for ks, kv in vc.items():
                        if known.get(ks, -1) < kv:
                            known[ks] = kv
                if known.get(sv[0], -1) < sv[1]:
                    known[sv[0]] = sv[1]
            if len(kept) != len(resolved) and \
                    type(inst).__name__ != "InstDrain":
                si.on_wait = [resolved[i][0] for i in kept]
                inst.sync_info = si
            for u in si.on_update:
                if (
                    u.update_mode not in ("sem-inc", "sem-add-imm")
                    or u.update_reg is not None
                    or u.ant_name in nonmono
                ):
                    continue
                s = u.ant_name
                cum[s] = cum.get(s, 0) + u.update_value
                vc_post = dict(known)
                vc_post[s] = max(vc_post.get(s, 0), cum[s])
                postings.setdefault(s, []).append((cum[s], vc_post))
                # NOTE: do NOT credit own ticks to the chain via program
                # order — engines are pipelined, so same-engine RAW hazards
                # legitimately need their self-waits (deep-pipeline rule).
    if verbose:
        print(f"_reduce_waits: dropped {n_drop} waits")
    return n_drop


def _weights2():
    """bf16 hi/lo split weights for the K=128-packed scheme.

    Coefficient rows m=1..64 go into the matmul (the m=0 row of V is the
    constant r_c^64 per circle — after column scaling its coefficient is
    exactly 1.0 on re-columns for x_real[:,0] and on im-columns for
    x_imag[:,0], handled by a tiny K=4 matmul with an exact 0/1 pattern).
    Returns WHI/WLO (128, 256) bf16-in-f32, E4 (4, 256), s^2."""
    import ml_dtypes

    W_R, W_I, s2 = _weights()
    # rows m=1..64 stacked: [W_R[1:65] ; W_I[1:65]] -> (128, 256)
    w = np.concatenate([W_R[1:], W_I[1:]], axis=0).astype(np.float32)
    whi = w.astype(ml_dtypes.bfloat16)
    wlo = (w - whi.astype(np.float32)).astype(ml_dtypes.bfloat16)
    # x0 pattern: re-columns get xr0 (rows 0,1 = hi,lo), im-columns get xi0
    re_col = ((np.arange(NPTS) % 128) < K).astype(np.float32)
    e4 = np.stack([re_col, re_col, 1.0 - re_col, 1.0 - re_col]).astype(
        ml_dtypes.bfloat16
    )                                                    # (4, 256)
    return whi, wlo, e4, s2


def build_nc2(bc=BC, chunk=CHUNK, group=GROUP, reps=1,
              terms=4, do_epi=True, out_u8=False, psbufs=2, qbufs=2, mbufs=2):
    """bf16 split-precision variant: per tile, 4 accumulating matmuls
    (K=4 x0-term, hi@Whi, hi@Wlo, lo@Whi) at 1 cycle/row instead of fp32's
    4 cycles/row. Inputs are host-split into bf16 hi+lo (same total bytes
    as fp32) and packed [xr(64);xi(64)] so DMAs span all 128 partitions."""
    import concourse.bass as bass
    import concourse.tile as tile
    from concourse import mybir

    f32 = mybir.dt.float32
    bf16 = mybir.dt.bfloat16
    _, _, _, s2 = _weights2()

    tiles = bc // P
    assert tiles % chunk == 0 and chunk % group == 0
    ccols = chunk * P

    nc = bass.Bass()
    # per-chunk interleave [hi-chunk(128 rows) | lo-chunk], bf16
    xs = nc.declare_dram_parameter("xs", [P, 2 * bc], bf16, isOutput=False)
    # x0 splits: rows xr0_hi, xr0_lo, xi0_hi, xi0_lo
    x0 = nc.declare_dram_parameter("x0", [4, bc], bf16, isOutput=False)
    # [WHI | WLO] (128, 512) bf16
    wb = nc.declare_dram_parameter("wb", [P, 2 * NPTS], bf16, isOutput=False)
    e4 = nc.declare_dram_parameter("e4", [4, NPTS], bf16, isOutput=False)
    out_dt = mybir.dt.uint8 if out_u8 else f32
    out = nc.declare_dram_parameter("out", [bc, K], out_dt, isOutput=True)

    with tile.TileContext(nc) as tc:
        with (
            tc.tile_pool(name="consts", bufs=1) as cpool,
            tc.tile_pool(name="xin", bufs=3) as xpool,
            tc.tile_pool(name="x0in", bufs=3) as zpool,
            tc.tile_pool(name="ps", bufs=psbufs, space="PSUM") as ppool,
            tc.tile_pool(name="q", bufs=qbufs) as qpool,
            tc.tile_pool(name="m", bufs=mbufs) as mpool,
        ):
            w_sb = cpool.tile([P, 2 * NPTS], bf16, tag="w")
            nc.sync.dma_start(w_sb[:], wb[:, :])
            e4_sb = cpool.tile([4, NPTS], bf16, tag="e4")
            nc.sync.dma_start(e4_sb[:], e4[:, :])
            whi_sb = w_sb[:, 0:NPTS]
            wlo_sb = w_sb[:, NPTS:2 * NPTS]

            # warm-up matmuls: absorb the const-DMA waits (1-wait HW limit)
            wps = ppool.tile([1, 2], f32, tag="ps")
            nc.tensor.matmul(
                wps[:], w_sb[:, 0:1], w_sb[:, 0:2], start=True, stop=True
            )
            nc.tensor.matmul(
                wps[:], e4_sb[:, 0:1], e4_sb[:, 0:2], start=True, stop=True
            )

            for c in [cc for _ in range(reps) for cc in range(tiles // chunk)]:
                x_sb = xpool.tile([P, 2 * ccols], bf16, tag="x")
                nc.sync.dma_start(
                    x_sb[:], xs[:, c * 2 * ccols:(c + 1) * 2 * ccols]
                )
                z_sb = zpool.tile([4, ccols], bf16, tag="z")
                nc.sync.dma_start(z_sb[:], x0[:, c * ccols:(c + 1) * ccols])

                m_sb = mpool.tile([P, chunk * K], out_dt, tag="m")

                for g in range(chunk // group):
                    ps = ppool.tile([P, group * NPTS], f32, tag="ps")
                    # dummy: absorbs the psum-reuse (WAR) wait
                    nc.tensor.matmul(
                        ps[0:1, 0:2], w_sb[:, 0:1], w_sb[:, 0:2],
                        start=True, stop=True,
                    )
                    for t in range(group):
                        col = (g * group + t) * P
                        pslice = ps[:, t * NPTS:(t + 1) * NPTS]
                        xhi = x_sb[:, col:col + P]
                        xlo = x_sb[:, ccols + col:ccols + col + P]
                        mms = [(z_sb[:, col:col + P], e4_sb[:]),
                               (xhi, whi_sb), (xhi, wlo_sb), (xlo, whi_sb)]
                        mms = mms[4 - terms:]
                        for i, (lhs, rhs) in enumerate(mms):
                            nc.tensor.matmul(
                                pslice, lhs, rhs,
                                start=(i == 0), stop=(i == len(mms) - 1),
                            )

                    # epilogue identical to v1
                    if not do_epi:
                        continue
                    q_sb = qpool.tile([P, group * NPTS + 1], f32, tag="q")
                    nc.scalar.copy(q_sb[0:1, group * NPTS:], w_sb[0:1, 0:1])
                    nc.scalar.activation(
                        q_sb[:, 0:group * NPTS], ps[:],
                        mybir.ActivationFunctionType.Square,
                    )
                    q4 = q_sb[:, 0:group * NPTS].rearrange(
                        "p (t c i) -> p t c i", t=group, c=2
                    )
                    tq = qpool.tile([P, group * P], f32, tag="tq")
                    nc.vector.tensor_tensor(
                        tq[:], q4[:, :, :, 0:K], q4[:, :, :, K:2 * K],
                        mybir.AluOpType.add,
                    )
                    tq3 = tq[:].rearrange("p (t c j) -> p t c j", t=group, c=2)
                    msl = m_sb[:, g * group * K:(g + 1) * group * K]
                    nc.vector.scalar_tensor_tensor(
                        msl.rearrange("p (t j) -> p t j", j=K),
                        tq3[:, :, 1, :], s2, tq3[:, :, 0, :],
                        mybir.AluOpType.mult, mybir.AluOpType.is_le,
                    )

                if do_epi:
                    # batch rows are chunk-permuted (host packs column
                    # t*128+p = batch p*chunk+t) so each partition's chunk
                    # output is `chunk` consecutive DRAM rows -> >=512B
                    # contiguous per DMA descriptor (avoids HBM RMW)
                    oview = out[c * chunk * P:(c + 1) * chunk * P, :].rearrange(
                        "(p t) j -> p t j", t=chunk
                    )
                    nc.sync.dma_start(
                        oview, m_sb[:].rearrange("p (t j) -> p t j", j=K)
                    )

    _reduce_waits(nc)
    _legalize_waits(nc)
    return nc


def _pack_x2(xr, xi, chunk=CHUNK):
    """Host prep for build_nc2 from natural (bc, 65) f32 slices.

    Returns xs (128, 2*bc) bf16 with per-chunk [hi | lo] blocks where
    rows 0:64 = xr coeffs m=1..64 transposed, rows 64:128 = xi coeffs;
    and x0 (4, bc) bf16 rows xr0_hi, xr0_lo, xi0_hi, xi0_lo."""
    import ml_dtypes

    bf = ml_dtypes.bfloat16
    bcc = xr.shape[0]
    coef = np.concatenate([xr[:, 1:].T, xi[:, 1:].T], axis=0)  # (128, bc) f32
    ccols = chunk * P
    nch = bcc // ccols
    # per-chunk column permutation: position t*128+p holds batch p*chunk+t
    # (so the output DMA writes `chunk` consecutive rows per partition)
    coef = (
        coef.reshape(P, nch, P, chunk).transpose(0, 1, 3, 2).reshape(P, bcc)
    )
    hi = coef.astype(bf)
    lo = (coef - hi.astype(np.float32)).astype(bf)
    hi3 = hi.reshape(P, nch, ccols)
    lo3 = lo.reshape(P, nch, ccols)
    xs = np.stack([hi3, lo3], axis=2).reshape(P, 2 * bcc)

    x0c = np.stack([xr[:, 0], xi[:, 0]]).astype(np.float32)    # (2, bc)
    x0c = (
        x0c.reshape(2, nch, P, chunk).transpose(0, 1, 3, 2).reshape(2, bcc)
    )
    h0 = x0c.astype(bf)
    l0 = (x0c - h0.astype(np.float32)).astype(bf)
    x0 = np.stack([h0[0], l0[0], h0[1], l0[1]])                # (4, bc)
    return np.ascontiguousarray(xs), np.ascontiguousarray(x0)


def _get_nc():
    if "nc" not in _CACHE:
        _CACHE["nc"] = build_nc2(out_u8=True, qbufs=3, mbufs=3)
    return _CACHE["nc"]


def _pack_x(xrt_c, xit_c, chunk=CHUNK):
    """Interleave xr/xi chunks: [65, BC]x2 -> [65, 2*BC] per-chunk blocks."""
    kp1, bcc = xrt_c.shape
    ccols = chunk * P
    nchunks = bcc // ccols
    xr3 = xrt_c.reshape(kp1, nchunks, ccols)
    xi3 = xit_c.reshape(kp1, nchunks, ccols)
    return np.stack([xr3, xi3], axis=2).reshape(kp1, 2 * bcc)


def make_in_maps(x_real, x_imag):
    whi, wlo, e4, _ = _weights2()
    wb = np.ascontiguousarray(
        np.concatenate([whi, wlo], axis=1)
    )                                                # (128, 512) bf16
    xr = np.asarray(x_real, dtype=np.float32)
    xi = np.asarray(x_imag, dtype=np.float32)
    in_maps = []
    for i in range(NCORES):
        xs, x0 = _pack_x2(xr[i * BC:(i + 1) * BC], xi[i * BC:(i + 1) * BC])
        in_maps.append({"xs": xs, "x0": x0, "wb": wb, "e4": e4})
    return in_maps


def kernel(x_real, x_imag):
    from concourse.bass_utils import run_bass_kernel_spmd

    in_maps = make_in_maps(x_real, x_imag)
    res = run_bass_kernel_spmd(_get_nc(), in_maps, list(range(NCORES)))
    # device emits uint8 0/1; the reference dtype is float32
    return np.concatenate(
        [res.results[i]["out"] for i in range(NCORES)], axis=0
    ).astype(np.float32)


# revision 30
# speedup vs baseline: 774.7817x; 2.1162x over previous
"""Trainium2 Bass kernel for nn_Decoder_70781061038951.

Math: for each row b of (B, 65) complex coefficients x = x_real + i*x_imag
(highest degree first), evaluate p(z) at 128 fixed points (64 roots-of-unity
angles on circle radius 1/r and 64 on radius r, r = sqrt(1+sin(pi/64))),
then output m[b, j] = (r^64 * |p(z0_j)| >= |p(z1_j)|) as float32 (B, 64).

Reformulation: polynomial evaluation at fixed points == complex Vandermonde
matmul res = X @ V, decomposed into two real PE matmuls per batch tile:
    psum[b, c] = sum_m XrT[m, b] * W_R[m, c] + XiT[m, b] * W_I[m, c]
with psum column layout [re0(64) | im0(64) | re1(64) | im1(64)] per tile.
Circle-0 columns of V are pre-scaled by s = r^64 and circle-1 by 1/s, so the
compare is  t0 >= s^2 * t1  on squared magnitudes (no sqrt).

Inputs are transposed on the host to [65, B] so the contraction dim lands on
SBUF partitions without any on-device transposes; xr/xi chunks are interleaved
into one array so each chunk is ONE contiguous DMA (the PE Matmult instruction
supports a single sync-wait condition, so every matmul may depend on at most
one new semaphore; tiny warm-up/dummy matmuls pre-absorb the other waits).
Batch is sharded across the 8 NeuronCores (pure data parallel).
"""

import numpy as np

B = 262144
K = 64
KP1 = 65
NCORES = 8
BC = B // NCORES          # 32768 rows per core
P = 128                   # batch tile (psum partitions)
NPTS = 256                # psum cols per tile: re0|im0|re1|im1
TILES = BC // P           # 256
CHUNK = 16                # batch tiles per input DMA chunk
GROUP = 8                 # batch tiles per psum group (4 banks)

_CACHE = {}


def _weights():
    """Host-side Vandermonde blocks W_R, W_I (65, 256) float32 and s^2."""
    r = float(np.sqrt(1.0 + np.sin(np.pi / K)))
    s = r ** K
    ang = 2.0 * np.pi * np.arange(K) / K
    z0 = (1.0 / r) * np.exp(1j * ang)
    z1 = r * np.exp(1j * ang)
    z = np.concatenate([z0, z1])                     # (128,) complex128
    expo = K - np.arange(KP1)                        # (65,) degree of coeff m
    V = z[None, :] ** expo[:, None]                  # (65, 128)
    col_scale = np.where(np.arange(2 * K) < K, s, 1.0 / s)
    Vs = V * col_scale[None, :]
    re, im = Vs.real, Vs.imag
    W_R = np.concatenate(
        [re[:, :K], im[:, :K], re[:, K:], im[:, K:]], axis=1
    ).astype(np.float32)                             # (65, 256)
    W_I = np.concatenate(
        [-im[:, :K], re[:, :K], -im[:, K:], re[:, K:]], axis=1
    ).astype(np.float32)
    return W_R, W_I, float(s * s)


def build_nc(bc=BC, chunk=CHUNK, group=GROUP, reps=1,
             do_mm=True, do_epi=True, do_out=True):
    """Build the single-core Bass program (same program for all 8 cores).

    reps > 1 re-runs the whole computation (idempotent) inside one NEFF —
    used to measure device time differentially under the axon dispatch
    overhead: T(R) - T(1) = (R-1) * device_time."""
    import concourse.bass as bass
    import concourse.tile as tile
    from concourse import mybir

    f32 = mybir.dt.float32
    _, _, s2 = _weights()

    tiles = bc // P
    assert tiles % chunk == 0 and chunk % group == 0
    ccols = chunk * P

    nc = bass.Bass()
    # xb: per-chunk interleave [xr chunk | xi chunk], each chunk one DMA
    xb = nc.declare_dram_parameter("xb", [KP1, 2 * bc], f32, isOutput=False)
    # wb: [W_R | W_I] (65, 512)
    wb = nc.declare_dram_parameter("wb", [KP1, 2 * NPTS], f32, isOutput=False)
    out = nc.declare_dram_parameter("out", [bc, K], f32, isOutput=True)

    with tile.TileContext(nc) as tc:
        with (
            tc.tile_pool(name="consts", bufs=1) as cpool,
            tc.tile_pool(name="xin", bufs=3) as xpool,
            tc.tile_pool(name="ps", bufs=2, space="PSUM") as ppool,
            tc.tile_pool(name="q", bufs=2) as qpool,
            tc.tile_pool(name="m", bufs=2) as mpool,
        ):
            w_sb = cpool.tile([KP1, 2 * NPTS], f32, tag="w")
            nc.sync.dma_start(w_sb[:], wb[:, :])
            wr_sb = w_sb[:, 0:NPTS]
            wi_sb = w_sb[:, NPTS:2 * NPTS]

            # warm-up matmul: absorbs the w-DMA wait so real matmuls never
            # wait on more than one semaphore (PE HW limit: 1 sync wait)
            wps = ppool.tile([1, 2], f32, tag="ps")
            nc.tensor.matmul(
                wps[:], w_sb[:, 0:1], w_sb[:, 0:2], start=True, stop=True
            )

            for c in [cc for _ in range(reps) for cc in range(tiles // chunk)]:
                x_sb = xpool.tile([KP1, 2 * ccols], f32, tag="x")
                nc.sync.dma_start(
                    x_sb[:], xb[:, c * 2 * ccols:(c + 1) * 2 * ccols]
                )

                m_sb = mpool.tile([P, chunk * K], f32, tag="m")

                for g in range(chunk // group) if do_mm else []:
                    ps = ppool.tile([P, group * NPTS], f32, tag="ps")
                    # dummy matmul: absorbs the psum-reuse (WAR) wait
                    nc.tensor.matmul(
                        ps[0:1, 0:2], w_sb[:, 0:1], w_sb[:, 0:2],
                        start=True, stop=True,
                    )
                    for t in range(group):
                        col = (g * group + t) * P
                        pslice = ps[:, t * NPTS:(t + 1) * NPTS]
                        nc.tensor.matmul(
                            pslice, x_sb[:, col:col + P], wr_sb,
                            start=True, stop=False,
                        )
                        nc.tensor.matmul(
                            pslice, x_sb[:, ccols + col:ccols + col + P], wi_sb,
                            start=False, stop=True,
                        )

                    if not do_epi:
                        continue
                    # q = psum^2 (one big ACT op over the whole group).
                    # dummy copy into a scratch column first: absorbs the
                    # q-slot WAR (DVE) wait so the Square itself carries only
                    # the PE data-ready wait (one sync-wait slot per instr).
                    q_sb = qpool.tile([P, group * NPTS + 1], f32, tag="q")
                    nc.scalar.copy(q_sb[0:1, group * NPTS:], w_sb[0:1, 0:1])
                    nc.scalar.activation(
                        q_sb[:, 0:group * NPTS], ps[:],
                        mybir.ActivationFunctionType.Square,
                    )
                    # tq = re^2 + im^2 ; layout per tile [t0(64) | t1(64)]
                    q4 = q_sb[:, 0:group * NPTS].rearrange(
                        "p (t c i) -> p t c i", t=group, c=2
                    )
                    tq = qpool.tile([P, group * P], f32, tag="tq")
                    nc.vector.tensor_tensor(
                        tq[:], q4[:, :, :, 0:K], q4[:, :, :, K:2 * K],
                        mybir.AluOpType.add,
                    )
                    # m = (s^2 * t1 <= t0)
                    tq3 = tq[:].rearrange("p (t c j) -> p t c j", t=group, c=2)
                    msl = m_sb[:, g * group * K:(g + 1) * group * K]
                    nc.vector.scalar_tensor_tensor(
                        msl.rearrange("p (t j) -> p t j", j=K),
                        tq3[:, :, 1, :], s2, tq3[:, :, 0, :],
                        mybir.AluOpType.mult, mybir.AluOpType.is_le,
                    )

                if do_out and do_mm and do_epi:
                    oview = out[c * chunk * P:(c + 1) * chunk * P, :].rearrange(
                        "(t p) j -> p t j", p=P
                    )
                    nc.sync.dma_start(
                        oview, m_sb[:].rearrange("p (t j) -> p t j", j=K)
                    )

    _reduce_waits(nc)
    _legalize_waits(nc)
    return nc


def _legalize_waits(nc):
    """The TPB instruction encodings carry at most ONE sync-wait. After
    transitive reduction the only residual multi-wait instruction is the
    kernel-tail drain (waits on every DMA lane + engine sem). Split its
    waits onto injected same-engine NoOps placed immediately before it."""
    from concourse import mybir

    for blk in nc.m.functions[0].blocks:
        while True:
            target = None
            for idx, inst in enumerate(blk.instructions):
                si = getattr(inst, "sync_info", None)
                if si is not None and len(si.on_wait) > 1:
                    target = (idx, inst, si)
                    break
            if target is None:
                break
            idx, inst, si = target
            waits = list(si.on_wait)
            for w in waits[:-1]:
                nop = mybir.InstNoOp(
                    name=nc.get_next_instruction_name(), ins=[], outs=[]
                )
                nop.engine = inst.engine
                nop.sync_info = mybir.SyncInfo(on_wait=[w], on_update=[])
                nc.register_instruction(nop)
                blk.instructions.insert(idx, nop)
                idx += 1
            si.on_wait = [waits[-1]]
            inst.sync_info = si


def _reduce_waits(nc, verbose=False):
    """Transitive sync-wait reduction. Tile's add_semaphores pass is minimal
    per-instruction but NOT transitively minimal across engines (documented),
    while the HW instruction encodings have very few wait slots (Matmult: 1).
    Vector-clock pass: a wait (S >= v) is dropped when the knowledge chain of
    the instruction's engine already implies it — via program order on the
    engine's own FIFO queue or via the dispatch-time knowledge of whichever
    instruction posted tick v (completion publishes its knowledge).
    For DMA-lane sems (possible out-of-order completion across rings), tick v
    is conservatively credited with only the knowledge of the k-th posting in
    dispatch order (k = postings needed to reach v)."""
    # engines whose instructions retire in order (self-waits implied)
    fifo_self = {"PE", "Activation", "DVE", "Pool", "SP"}
    # sems that are ever updated by anything other than a plain increment
    # (barrier sems use sem-sub) are non-monotonic: leave them untouched
    nonmono = set()
    for blk in nc.m.functions[0].blocks:
        for inst in blk.instructions:
            si = getattr(inst, "sync_info", None)
            if si is None:
                continue
            for u in si.on_update:
                if u.update_mode not in ("sem-inc", "sem-add-imm") \
                        or u.update_reg is not None:
                    nonmono.add(u.ant_name)
    chain_known = {}   # engine -> {sem_name: guaranteed_value}
    postings = {}      # sem_name -> list of (abs_value, vc_dict)
    cum = {}           # sem_name -> cumulative value
    n_drop = 0
    for blk in nc.m.functions[0].blocks:
        for inst in blk.instructions:
            eng = str(getattr(inst, "engine", "?")).split(".")[-1]
            si = getattr(inst, "sync_info", None)
            if si is None:
                continue
            known = chain_known.setdefault(eng, {})
            # resolve every wait to (sem, value, poster-VC); None = untouchable
            resolved = []
            for w in si.on_wait:
                if (
                    w.wait_mode != "sem-ge-imm"
                    or w.wait_reg is not None
                    or w.ant_name in nonmono
                ):
                    resolved.append((w, None, None))
                    continue
                s, v = w.ant_name, w.wait_value
                vc = None
                for val, pvc in postings.get(s, ()):
                    if val >= v:
                        vc = pvc
                        break
                resolved.append((w, (s, v), vc))
            # drop waits implied by chain knowledge + the other kept waits
            kept = list(range(len(resolved)))
            changed = True
            while changed:
                changed = False
                for i in list(kept):
                    _, sv, _ = resolved[i]
                    if sv is None:
                        continue
                    k = dict(known)
                    for j in kept:
                        if j == i:
                            continue
                        _, svj, vcj = resolved[j]
                        if svj is not None:
                            sj, vj = svj
                            if k.get(sj, -1) < vj:
                                k[sj] = vj
                        if vcj:
                            for ks, kv in vcj.items():
                                if k.get(ks, -1) < kv:
                                    k[ks] = kv
                    if k.get(sv[0], -1) >= sv[1]:
                        kept.remove(i)
                        n_drop += 1
                        changed = True
            # merge kept waits into chain knowledge
            for i in kept:
                _, sv, vc = resolved[i]
                if sv is None:
                    continue
                if vc:
                    for ks, kv in vc.items():
                        if known.get(ks, -1) < kv:
                            known[ks] = kv
                if known.get(sv[0], -1) < sv[1]:
                    known[sv[0]] = sv[1]
            if len(kept) != len(resolved) and \
                    type(inst).__name__ != "InstDrain":
                si.on_wait = [resolved[i][0] for i in kept]
                inst.sync_info = si
            for u in si.on_update:
                if (
                    u.update_mode not in ("sem-inc", "sem-add-imm")
                    or u.update_reg is not None
                    or u.ant_name in nonmono
                ):
                    continue
                s = u.ant_name
                cum[s] = cum.get(s, 0) + u.update_value
                vc_post = dict(known)
                vc_post[s] = max(vc_post.get(s, 0), cum[s])
                postings.setdefault(s, []).append((cum[s], vc_post))
                # NOTE: do NOT credit own ticks to the chain via program
                # order — engines are pipelined, so same-engine RAW hazards
                # legitimately need their self-waits (deep-pipeline rule).
    if verbose:
        print(f"_reduce_waits: dropped {n_drop} waits")
    return n_drop


def _weights2():
    """bf16 hi/lo split weights for the K=128-packed scheme.

    Coefficient rows m=1..64 go into the matmul (the m=0 row of V is the
    constant r_c^64 per circle — after column scaling its coefficient is
    exactly 1.0 on re-columns for x_real[:,0] and on im-columns for
    x_imag[:,0], handled by a tiny K=4 matmul with an exact 0/1 pattern).
    Returns WHI/WLO (128, 256) bf16-in-f32, E4 (4, 256), s^2."""
    import ml_dtypes

    W_R, W_I, s2 = _weights()
    # rows m=1..64 stacked: [W_R[1:65] ; W_I[1:65]] -> (128, 256)
    w = np.concatenate([W_R[1:], W_I[1:]], axis=0).astype(np.float32)
    whi = w.astype(ml_dtypes.bfloat16)
    wlo = (w - whi.astype(np.float32)).astype(ml_dtypes.bfloat16)
    # x0 pattern: re-columns get xr0 (rows 0,1 = hi,lo), im-columns get xi0.
    # Padded to 128 rows of zeros: K<32 matmuls measured ~6x slower than
    # K=128 on this toolchain, so the x0 matmul runs at K=128 against a
    # zero-padded stationary (device memsets the pad rows).
    re_col = ((np.arange(NPTS) % 128) < K).astype(np.float32)
    e4 = np.zeros((P, NPTS), np.float32)
    e4[0] = re_col
    e4[1] = re_col
    e4[2] = 1.0 - re_col
    e4[3] = 1.0 - re_col
    e4 = e4.astype(ml_dtypes.bfloat16)                   # (128, 256)
    return whi, wlo, e4, s2


def build_nc2(bc=BC, chunk=CHUNK, group=GROUP, reps=1,
              terms=4, do_epi=True, out_u8=False, psbufs=2, qbufs=2, mbufs=2):
    """bf16 split-precision variant: per tile, 4 accumulating matmuls
    (K=4 x0-term, hi@Whi, hi@Wlo, lo@Whi) at 1 cycle/row instead of fp32's
    4 cycles/row. Inputs are host-split into bf16 hi+lo (same total bytes
    as fp32) and packed [xr(64);xi(64)] so DMAs span all 128 partitions."""
    import concourse.bass as bass
    import concourse.tile as tile
    from concourse import mybir

    f32 = mybir.dt.float32
    bf16 = mybir.dt.bfloat16
    _, _, _, s2 = _weights2()

    tiles = bc // P
    assert tiles % chunk == 0 and chunk % group == 0
    ccols = chunk * P

    nc = bass.Bass()
    # per-chunk interleave [hi-chunk(128 rows) | lo-chunk], bf16
    xs = nc.declare_dram_parameter("xs", [P, 2 * bc], bf16, isOutput=False)
    # x0 splits: rows xr0_hi, xr0_lo, xi0_hi, xi0_lo
    x0 = nc.declare_dram_parameter("x0", [32, bc], bf16, isOutput=False)
    # [WHI | WLO] (128, 512) bf16
    wb = nc.declare_dram_parameter("wb", [P, 2 * NPTS], bf16, isOutput=False)
    e4 = nc.declare_dram_parameter("e4", [P, NPTS], bf16, isOutput=False)
    out_dt = mybir.dt.uint8 if out_u8 else f32
    out = nc.declare_dram_parameter("out", [bc, K], out_dt, isOutput=True)

    with tile.TileContext(nc) as tc:
        with (
            tc.tile_pool(name="consts", bufs=1) as cpool,
            tc.tile_pool(name="xin", bufs=3) as xpool,
            tc.tile_pool(name="x0in", bufs=3) as zpool,
            tc.tile_pool(name="ps", bufs=psbufs, space="PSUM") as ppool,
            tc.tile_pool(name="q", bufs=qbufs) as qpool,
            tc.tile_pool(name="m", bufs=mbufs) as mpool,
        ):
            w_sb = cpool.tile([P, 2 * NPTS], bf16, tag="w")
            nc.sync.dma_start(w_sb[:], wb[:, :])
            e4_sb = cpool.tile([P, NPTS], bf16, tag="e4")
            nc.sync.dma_start(e4_sb[:], e4[:, :])
            whi_sb = w_sb[:, 0:NPTS]
            wlo_sb = w_sb[:, NPTS:2 * NPTS]

            # warm-up matmuls: absorb the const-DMA waits (1-wait HW limit)
            wps = ppool.tile([1, 2], f32, tag="ps")
            nc.tensor.matmul(
                wps[:], w_sb[:, 0:1], w_sb[:, 0:2], start=True, stop=True
            )
            nc.tensor.matmul(
                wps[:], e4_sb[:, 0:1], e4_sb[:, 0:2], start=True, stop=True
            )

            for c in [cc for _ in range(reps) for cc in range(tiles // chunk)]:
                x_sb = xpool.tile([P, 2 * ccols], bf16, tag="x")
                nc.sync.dma_start(
                    x_sb[:], xs[:, c * 2 * ccols:(c + 1) * 2 * ccols]
                )
                z_sb = zpool.tile([P, ccols], bf16, tag="z")
                nc.sync.dma_start(z_sb[0:32, :], x0[:, c * ccols:(c + 1) * ccols])
                nc.gpsimd.memset(z_sb[32:64, :], 0.0)
                nc.gpsimd.memset(z_sb[64:P, :], 0.0)

                m_sb = mpool.tile([P, chunk * K], out_dt, tag="m")

                for g in range(chunk // group):
                    ps = ppool.tile([P, group * NPTS], f32, tag="ps")
                    # dummy: absorbs the psum-reuse (WAR) wait
                    nc.tensor.matmul(
                        ps[0:1, 0:2], w_sb[:, 0:1], w_sb[:, 0:2],
                        start=True, stop=True,
                    )
                    for t in range(group):
                        col = (g * group + t) * P
                        pslice = ps[:, t * NPTS:(t + 1) * NPTS]
                        xhi = x_sb[:, col:col + P]
                        xlo = x_sb[:, ccols + col:ccols + col + P]
                        mms = [(z_sb[:, col:col + P], e4_sb[:]),
                               (xhi, whi_sb), (xhi, wlo_sb), (xlo, whi_sb)]
                        mms = mms[4 - terms:]
                        for i, (lhs, rhs) in enumerate(mms):
                            nc.tensor.matmul(
                                pslice, lhs, rhs,
                                start=(i == 0), stop=(i == len(mms) - 1),
                            )

                    # epilogue identical to v1
                    if not do_epi:
                        continue
                    q_sb = qpool.tile([P, group * NPTS + 1], f32, tag="q")
                    nc.scalar.copy(q_sb[0:1, group * NPTS:], w_sb[0:1, 0:1])
                    nc.scalar.activation(
                        q_sb[:, 0:group * NPTS], ps[:],
                        mybir.ActivationFunctionType.Square,
                    )
                    q4 = q_sb[:, 0:group * NPTS].rearrange(
                        "p (t c i) -> p t c i", t=group, c=2
                    )
                    tq = qpool.tile([P, group * P], f32, tag="tq")
                    nc.vector.tensor_tensor(
                        tq[:], q4[:, :, :, 0:K], q4[:, :, :, K:2 * K],
                        mybir.AluOpType.add,
                    )
                    tq3 = tq[:].rearrange("p (t c j) -> p t c j", t=group, c=2)
                    msl = m_sb[:, g * group * K:(g + 1) * group * K]
                    nc.vector.scalar_tensor_tensor(
                        msl.rearrange("p (t j) -> p t j", j=K),
                        tq3[:, :, 1, :], s2, tq3[:, :, 0, :],
                        mybir.AluOpType.mult, mybir.AluOpType.is_le,
                    )

                if do_epi:
                    # batch rows are chunk-permuted (host packs column
                    # t*128+p = batch p*chunk+t) so each partition's chunk
                    # output is `chunk` consecutive DRAM rows -> >=512B
                    # contiguous per DMA descriptor (avoids HBM RMW)
                    oview = out[c * chunk * P:(c + 1) * chunk * P, :].rearrange(
                        "(p t) j -> p t j", t=chunk
                    )
                    nc.sync.dma_start(
                        oview, m_sb[:].rearrange("p (t j) -> p t j", j=K)
                    )

    _reduce_waits(nc)
    _legalize_waits(nc)
    return nc


def _pack_x2(xr, xi, chunk=CHUNK):
    """Host prep for build_nc2 from natural (bc, 65) f32 slices.

    Returns xs (128, 2*bc) bf16 with per-chunk [hi | lo] blocks where
    rows 0:64 = xr coeffs m=1..64 transposed, rows 64:128 = xi coeffs;
    and x0 (4, bc) bf16 rows xr0_hi, xr0_lo, xi0_hi, xi0_lo."""
    import ml_dtypes

    bf = ml_dtypes.bfloat16
    bcc = xr.shape[0]
    coef = np.concatenate([xr[:, 1:].T, xi[:, 1:].T], axis=0)  # (128, bc) f32
    ccols = chunk * P
    nch = bcc // ccols
    # per-chunk column permutation: position t*128+p holds batch p*chunk+t
    # (so the output DMA writes `chunk` consecutive rows per partition)
    coef = (
        coef.reshape(P, nch, P, chunk).transpose(0, 1, 3, 2).reshape(P, bcc)
    )
    hi = coef.astype(bf)
    lo = (coef - hi.astype(np.float32)).astype(bf)
    hi3 = hi.reshape(P, nch, ccols)
    lo3 = lo.reshape(P, nch, ccols)
    xs = np.stack([hi3, lo3], axis=2).reshape(P, 2 * bcc)

    x0c = np.stack([xr[:, 0], xi[:, 0]]).astype(np.float32)    # (2, bc)
    x0c = (
        x0c.reshape(2, nch, P, chunk).transpose(0, 1, 3, 2).reshape(2, bcc)
    )
    h0 = x0c.astype(bf)
    l0 = (x0c - h0.astype(np.float32)).astype(bf)
    x0 = np.zeros((32, bcc), dtype=bf)                         # rows 4:32 zero
    x0[0], x0[1], x0[2], x0[3] = h0[0], l0[0], h0[1], l0[1]
    return np.ascontiguousarray(xs), np.ascontiguousarray(x0)


def _get_nc():
    if "nc" not in _CACHE:
        _CACHE["nc"] = build_nc2(out_u8=True, qbufs=3, mbufs=3)
    return _CACHE["nc"]


def _pack_x(xrt_c, xit_c, chunk=CHUNK):
    """Interleave xr/xi chunks: [65, BC]x2 -> [65, 2*BC] per-chunk blocks."""
    kp1, bcc = xrt_c.shape
    ccols = chunk * P
    nchunks = bcc // ccols
    xr3 = xrt_c.reshape(kp1, nchunks, ccols)
    xi3 = xit_c.reshape(kp1, nchunks, ccols)
    return np.stack([xr3, xi3], axis=2).reshape(kp1, 2 * bcc)


def make_in_maps(x_real, x_imag):
    whi, wlo, e4, _ = _weights2()
    wb = np.ascontiguousarray(
        np.concatenate([whi, wlo], axis=1)
    )                                                # (128, 512) bf16
    xr = np.asarray(x_real, dtype=np.float32)
    xi = np.asarray(x_imag, dtype=np.float32)
    in_maps = []
    for i in range(NCORES):
        xs, x0 = _pack_x2(xr[i * BC:(i + 1) * BC], xi[i * BC:(i + 1) * BC])
        in_maps.append({"xs": xs, "x0": x0, "wb": wb, "e4": e4})
    return in_maps


def kernel(x_real, x_imag):
    from concourse.bass_utils import run_bass_kernel_spmd

    in_maps = make_in_maps(x_real, x_imag)
    res = run_bass_kernel_spmd(_get_nc(), in_maps, list(range(NCORES)))
    # device emits uint8 0/1; the reference dtype is float32
    return np.concatenate(
        [res.results[i]["out"] for i in range(NCORES)], axis=0
    ).astype(np.float32)
